# revision 9
# baseline (speedup 1.0000x reference)
"""Trainium2 Bass kernel: 8-head causal MHA with RoPE, B=2 T=2048 E=1024 H=8 D=512.

Sharding: 8 cores = 2 (batch) x 4 (head-pair) tensor-parallel groups.
Each core computes q/k/v projections for its 2 heads, causal attention,
and a row-parallel partial of the output projection; the host sums the
4 partials per batch (unshard) and transposes back to [B, T, E].

All matmuls run in float32r (single-pass fp32, ~12-bit mantissa) on the PE.
RoPE's even/odd pairing is pre-baked into the wq/wk rows host-side (scores
are invariant under a shared permutation of the head dim), so on-chip RoPE
is 6 contiguous tensor-tensor ops per (t-block, pair-block). The 1/sqrt(D)
score scale is folded into wq host-side. Scores are computed transposed
(keys on partitions) so softmax probs feed attn@V and the wo matmul with
no on-chip transposes anywhere.
"""

import os
import sys

for _p in ("/opt/trn_rl_repo", "/root/.axon_site/_ro/trn_rl_repo"):
    if os.path.isdir(_p) and _p not in sys.path:
        sys.path.insert(0, _p)

import numpy as np

import concourse.bacc as bacc
import concourse.mybir as mybir
import concourse.tile as tile
from concourse.bass_utils import run_bass_kernel_spmd

B, T, E, H, D = 2, 2048, 1024, 8, 512
P = 128
NE = E // P          # 8 e-chunks (contraction)
NTB = T // 512       # 4 t-blocks of 512
NTT = T // P         # 16 t-tiles of 128
NDC = D // P         # 4 d-chunks per head
HPC = 2              # heads per core
G = 4                # head groups (cores per batch)

F32R = mybir.dt.float32r
F32 = mybir.dt.float32
F16 = mybir.dt.float16

MASK_NEG = -30000.0

# knobs
WS_BUFS = 2
BLK_BUFS = 5
# bisect knob: "full" | "proj0" (h0 projections only) | "attn0" (h0 proj+attn)
# | "head0" (h0 proj+attn+wo) | "noattn" (both heads proj only)
KVARIANT = os.environ.get("KVARIANT", "full")


def _build_nc():
    nc = bacc.Bacc("TRN2", target_bir_lowering=False, debug=False, num_devices=8)

    xT = nc.declare_dram_parameter("xT", [E, T], F32R, isOutput=False)
    wqT = nc.declare_dram_parameter("wqT", [E, HPC * D], F32R, isOutput=False)
    wkT = nc.declare_dram_parameter("wkT", [E, HPC * D], F32R, isOutput=False)
    wvT = nc.declare_dram_parameter("wvT", [E, HPC * D], F32R, isOutput=False)
    woT = nc.declare_dram_parameter("woT", [HPC * D, E], F32R, isOutput=False)
    cosT = nc.declare_dram_parameter("cosT", [D // 2, T], F16, isOutput=False)
    sinT = nc.declare_dram_parameter("sinT", [D // 2, T], F16, isOutput=False)
    masks = nc.declare_dram_parameter("masks", [4 * P, 512], F16, isOutput=False)
    ones = nc.declare_dram_parameter("ones", [P, 1], F32R, isOutput=False)
    outT = nc.declare_dram_parameter("outT", [E, T], F32, isOutput=True)

    # scratch for attention output (hd-major, rows hc*128+p with hc = h*4+dc)
    oT_dram = nc.dram_tensor("oT_scratch", [HPC * NDC, P, T], F32R)

    with tile.TileContext(nc) as tc:
        with (
            tc.tile_pool(name="glob", bufs=1) as gp,
            tc.tile_pool(name="left", bufs=1) as lp,
            tc.tile_pool(name="right", bufs=1, side="right") as rp,
            tc.tile_pool(name="psum", bufs=1, space="PSUM") as pp,
        ):
            # ---- persistent loads ----
            # NB: single DMAs >2MB hang the device; keep transfers <=1MB
            xt = lp.tile([P, NE, T], F32R, tag="xt")
            xr = xT.rearrange("(c p) t -> p c t", p=P)
            for c in range(NE):
                nc.sync.dma_start(xt[:, c, :], xr[:, c, :])
            cos_t = lp.tile([P, 2, T], F16, tag="cos")
            sin_t = lp.tile([P, 2, T], F16, tag="sin")
            nc.sync.dma_start(cos_t[:], cosT.rearrange("(d p) t -> p d t", p=P))
            nc.sync.dma_start(sin_t[:], sinT.rearrange("(d p) t -> p d t", p=P))
            mask_t = rp.tile([P, 4, 512], F16, tag="masks")
            nc.sync.dma_start(mask_t[:], masks.rearrange("(q p) c -> p q c", p=P))
            ones_t = gp.tile([P, 1], F32R, tag="ones")
            nc.sync.dma_start(ones_t[:], ones[:])

            n_heads = 1 if KVARIANT in ("proj0", "attn0", "head0") else HPC
            do_attn = KVARIANT not in ("proj0", "noattn")
            do_wo = KVARIANT in ("full", "head0")
            for h in range(n_heads):
                # ---- projections for head h ----
                qr = rp.tile([P, NDC, T], F32R, tag="qr", name=f"qr{h}")
                kr = rp.tile([P, NDC, T], F32R, tag="kr", name=f"kr{h}")
                vv = rp.tile([P, NTT, D], F32R, tag="vv", name=f"vv{h}")

                # q and k (rope'd, transposed [d', t] layout)
                for tname, wdram, dst in (("q", wqT, qr), ("k", wkT, kr)):
                    for dp in range(2):
                        ws = lp.tile([P, NE, 256], F32R, tag="ws", bufs=WS_BUFS,
                                     name=f"ws_{tname}{h}{dp}")
                        nc.sync.dma_start(
                            ws[:],
                            wdram.rearrange("(c p) d -> p c d", p=P)[
                                :, :, h * D + dp * 256 : h * D + (dp + 1) * 256
                            ],
                        )
                        for tb in range(NTB):
                            cols = slice(tb * 512, (tb + 1) * 512)
                            psA = pp.tile([P, 512], F32, tag="a", bufs=4,
                                          name=f"psA_{tname}{h}{dp}{tb}")
                            psB = pp.tile([P, 512], F32, tag="a", bufs=4,
                                          name=f"psB_{tname}{h}{dp}{tb}")
                            for c in range(NE):
                                nc.tensor.matmul(psA[:], ws[:, c, 0:128],
                                                 xt[:, c, cols],
                                                 start=(c == 0), stop=(c == NE - 1))
                            for c in range(NE):
                                nc.tensor.matmul(psB[:], ws[:, c, 128:256],
                                                 xt[:, c, cols],
                                                 start=(c == 0), stop=(c == NE - 1))
                            # rope: r1 = A*cos - B*sin -> dt=dp ; r2 = A*sin + B*cos -> dt=dp+2
                            ct = cos_t[:, dp, cols]
                            st = sin_t[:, dp, cols]
                            t_ac = gp.tile([P, 512], F32, tag="blk", bufs=BLK_BUFS,
                                           name=f"tac{h}{dp}{tb}{tname}")
                            t_bs = gp.tile([P, 512], F32, tag="blk", bufs=BLK_BUFS,
                                           name=f"tbs{h}{dp}{tb}{tname}")
                            nc.vector.tensor_mul(t_ac[:], psA[:], ct)
                            nc.vector.tensor_mul(t_bs[:], psB[:], st)
                            nc.vector.tensor_sub(dst[:, dp, cols], t_ac[:], t_bs[:])
                            t_as = gp.tile([P, 512], F32, tag="blk", bufs=BLK_BUFS,
                                           name=f"tas{h}{dp}{tb}{tname}")
                            t_bc = gp.tile([P, 512], F32, tag="blk", bufs=BLK_BUFS,
                                           name=f"tbc{h}{dp}{tb}{tname}")
                            nc.vector.tensor_mul(t_as[:], psA[:], st)
                            nc.vector.tensor_mul(t_bc[:], psB[:], ct)
                            nc.vector.tensor_add(dst[:, dp + 2, cols], t_as[:], t_bc[:])

                # v (natural [t, d] layout), by d-halves
                for dh in range(2):
                    ws = lp.tile([P, NE, 256], F32R, tag="ws", bufs=WS_BUFS,
                                 name=f"ws_v{h}{dh}")
                    nc.sync.dma_start(
                        ws[:],
                        wvT.rearrange("(c p) d -> p c d", p=P)[
                            :, :, h * D + dh * 256 : h * D + (dh + 1) * 256
                        ],
                    )
                    for tt in range(NTT):
                        psV = pp.tile([P, 256], F32, tag="b", bufs=2,
                                      name=f"psV{h}{dh}{tt}")
                        for c in range(NE):
                            nc.tensor.matmul(psV[:], xt[:, c, tt * P : (tt + 1) * P],
                                             ws[:, c, :],
                                             start=(c == 0), stop=(c == NE - 1))
                        nc.scalar.activation(vv[:, tt, dh * 256 : (dh + 1) * 256],
                                             psV[:],
                                             mybir.ActivationFunctionType.Copy)

                # ---- causal attention for head h ----
                for ib in range(NTB if do_attn else 0):
                    icols = slice(ib * 512, (ib + 1) * 512)
                    po = [pp.tile([P, 512], F32, tag="a", bufs=4, name=f"po{h}{ib}{dc}")
                          for dc in range(NDC)]
                    pd = pp.tile([1, 512], F32, tag="d", bufs=2, name=f"pd{h}{ib}")
                    jt_max = 4 * ib + 3
                    for jt in range(jt_max + 1):
                        ps = pp.tile([P, 512], F32, tag="b", bufs=2,
                                     name=f"ps{h}{ib}{jt}")
                        for dc in range(NDC):
                            nc.tensor.matmul(ps[:],
                                             kr[:, dc, jt * P : (jt + 1) * P],
                                             qr[:, dc, icols],
                                             start=(dc == 0), stop=(dc == NDC - 1))
                        q = jt - 4 * ib
                        if q >= 0:
                            nc.vector.tensor_add(ps[:], ps[:], mask_t[:, q, :])
                        e_t = gp.tile([P, 512], F32R, tag="blk", bufs=BLK_BUFS,
                                      name=f"et{h}{ib}{jt}")
                        nc.scalar.activation(e_t[:], ps[:],
                                             mybir.ActivationFunctionType.Exp)
                        for dc in range(NDC):
                            nc.tensor.matmul(po[dc][:],
                                             vv[:, jt, dc * P : (dc + 1) * P],
                                             e_t[:],
                                             start=(jt == 0), stop=(jt == jt_max))
                        nc.tensor.matmul(pd[:], ones_t[:], e_t[:],
                                         start=(jt == 0), stop=(jt == jt_max))
                    # normalize and spill oT block
                    d_sb = gp.tile([1, 512], F32, tag="blk", bufs=BLK_BUFS,
                                   name=f"dsb{h}{ib}")
                    nc.scalar.activation(d_sb[:], pd[:],
                                         mybir.ActivationFunctionType.Copy)
                    rb_d = gp.tile([P, 512], F32, tag="blk", bufs=BLK_BUFS,
                                   name=f"rbd{h}{ib}")
                    nc.gpsimd.partition_broadcast(rb_d[:], d_sb[:])
                    rb = gp.tile([P, 512], F32, tag="blk", bufs=BLK_BUFS,
                                 name=f"rb{h}{ib}")
                    nc.vector.reciprocal_approx_fast(rb[:], rb_d[:])
                    for dc in range(NDC):
                        stg = gp.tile([P, 512], F32R, tag="blk", bufs=BLK_BUFS,
                                      name=f"stg{h}{ib}{dc}")
                        nc.vector.tensor_mul(stg[:], po[dc][:], rb[:])
                        nc.sync.dma_start(oT_dram[h * NDC + dc, :, icols], stg[:])

            # ---- output projection (row-parallel partial) ----
            wo_t = rp.tile([P, NE, E], F32R, tag="qr", name="wo_t")
            wor = woT.rearrange("(c p) e -> p c e", p=P)
            for c in range(NE):
                nc.sync.dma_start(wo_t[:, c, :], wor[:, c, :])
            for tb in range(NTB if do_wo else 0):
                cols = slice(tb * 512, (tb + 1) * 512)
                ot_in = rp.tile([P, NE, 512], F32R, tag=("kr" if tb % 2 == 0 else "vv"),
                                name=f"ot_in{tb}")
                otr = oT_dram.rearrange("c p t -> p c t")[:, :, cols]
                for c in range(NE):
                    nc.sync.dma_start(ot_in[:, c, :], otr[:, c, :])
                for et in range(NE):
                    pw = pp.tile([P, 512], F32, tag="b", bufs=2, name=f"pw{tb}{et}")
                    for hc in range(NE):
                        nc.tensor.matmul(pw[:], wo_t[:, hc, et * P : (et + 1) * P],
                                         ot_in[:, hc, :],
                                         start=(hc == 0), stop=(hc == NE - 1))
                    ow = gp.tile([P, 512], F32, tag="blk", bufs=BLK_BUFS,
                                 name=f"ow{tb}{et}")
                    nc.scalar.activation(ow[:], pw[:],
                                         mybir.ActivationFunctionType.Copy)
                    nc.sync.dma_start(outT[et * P : (et + 1) * P, cols], ow[:])

    nc.compile()
    return nc


_NC = None


def _get_nc():
    global _NC
    if _NC is None:
        _NC = _build_nc()
    return _NC


def _prep_inputs(x, wq, wk, wv, wo):
    """Host-side shard prep. Returns in_maps list of 8 dicts (core = b*4+g)."""
    x = np.asarray(x, dtype=np.float32)
    wq = np.asarray(wq, dtype=np.float32)
    wk = np.asarray(wk, dtype=np.float32)
    wv = np.asarray(wv, dtype=np.float32)
    wo = np.asarray(wo, dtype=np.float32)

    # rope permutation of head-dim rows: per head, new order =
    # [pair-block 0 x1 | pair-block 0 x2 | pair-block 1 x1 | pair-block 1 x2]
    # i.e. d' = dp*256 + (0..127 -> even rows 2*(dp*128+i), 128..255 -> odd rows)
    perm = np.empty(D, dtype=np.int64)
    for dp in range(2):
        base = dp * 256
        pairs = dp * 128 + np.arange(128)
        perm[base : base + 128] = 2 * pairs          # x1
        perm[base + 128 : base + 256] = 2 * pairs + 1  # x2
    full_perm = np.concatenate([h * D + perm for h in range(H)])

    scale = 1.0 / np.sqrt(np.float32(D))
    wq_p = (wq[full_perm] * scale).astype(np.float32)
    wk_p = wk[full_perm].astype(np.float32)

    # rope tables [D/2, T] fp16 (pair-index major)
    inv_freq = 1.0 / (10000.0 ** (np.arange(0, D, 2, dtype=np.float64) / D))
    ang = inv_freq[:, None] * np.arange(T, dtype=np.float64)[None, :]
    cosT = np.cos(ang).astype(np.float16)
    sinT = np.sin(ang).astype(np.float16)

    # additive causal masks for the 4 diagonal sub-blocks [4*128, 512] fp16
    rj = np.arange(P)[:, None]
    c = np.arange(512)[None, :]
    masks = np.empty((4 * P, 512), dtype=np.float16)
    for q in range(4):
        masks[q * P : (q + 1) * P] = np.where(c >= 128 * q + rj, 0.0, MASK_NEG)

    ones = np.ones((P, 1), dtype=np.float32)

    in_maps = []
    for core in range(8):
        b, g = divmod(core, G)
        rows = slice(g * HPC * D, (g + 1) * HPC * D)
        in_maps.append({
            "xT": np.ascontiguousarray(x[b].T),
            "wqT": np.ascontiguousarray(wq_p[rows].T),
            "wkT": np.ascontiguousarray(wk_p[rows].T),
            "wvT": np.ascontiguousarray(wv[rows].T),
            "woT": np.ascontiguousarray(wo[:, rows].T),
            "cosT": cosT,
            "sinT": sinT,
            "masks": masks,
            "ones": ones,
        })
    return in_maps


def _assemble(results):
    """Sum the 4 TP partials per batch and transpose back to [B, T, E]."""
    out = np.empty((B, T, E), dtype=np.float32)
    for b in range(B):
        acc = results[b * G]["outT"].astype(np.float32)
        for g in range(1, G):
            acc = acc + results[b * G + g]["outT"]
        out[b] = acc.T
    return out


def kernel(x, wq, wk, wv, wo):
    nc = _get_nc()
    in_maps = _prep_inputs(x, wq, wk, wv, wo)
    res = run_bass_kernel_spmd(nc, in_maps, list(range(8)))
    return _assemble(res.results)
